# revision 19
# baseline (speedup 1.0000x reference)
"""Trainium2 Bass kernel for nn_ContextEncoder, v2.

Structure per core (J=128 sequences = 2 samples x 64 d):
  - feature: xs2[0:64, (t,b,d)] = tanh(Wt @ X.T + bt), bf16 matmuls,
    4-bank PSUM rounds with one FD=2048 tanh per round.
  - recurrence: per step, per dir: 4 hW matmuls N=128 into a 2-step PSUM
    group (double-buffered, xW for the next group prefetched between
    steps), one sigmoid FD=512, DVE chain of 4 ops using
    scalar_tensor_tensor fusions, one tanh FD=128.
    g-gate weights pre-scaled by 2 so tanh(g) = 2*sig(2g)-1; the cell is
      u2 = (sig2g - 0.5) * sig_i          (STT)
      c2 = sig_f * c_prev                 (TT)
      cn = 2*u2 + c2                      (STT)
      hn = sig_o * tanh(cn)               (TT)
  - h history goes to HT[j, t, 2h] via DMA xbar transposes (sync queue
    for dir 0, scalar queue for dir 1); no PE transposes, no PSUM slot.
  - tail: attention pooling (prod + pairwise trees + softmax + per-t
    weighted sum) and context norm via SEL matmuls.
"""

import sys

for _p in ("/opt/trn_rl_repo", "/root/.axon_site/_ro/trn_rl_repo"):
    if _p not in sys.path:
        sys.path.append(_p)

import numpy as np
import ml_dtypes

import concourse.bass as bass
import concourse.bacc as bacc
import concourse.tile as tile
from concourse import mybir
from concourse.bass_utils import run_bass_kernel_spmd

BF16NP = np.float16
F32 = mybir.dt.float32
BF16 = mybir.dt.float16
AF = mybir.ActivationFunctionType
ALU = mybir.AluOpType

B, T, D, NF = 16, 128, 64, 32
TS, H = 64, 128
NCORES = 8
BLOC = B // NCORES          # 2 samples per core
J = BLOC * D                # 128 sequences per core
R = J * T                   # 16384 (t, b, d) columns
G4 = 4 * H                  # 512 gates per direction
PERM = (0, 1, 2, 3)         # torch gate order kept as (i,f,g,o)
NORM_N = D * 2 * H          # 16384 context-norm elements per sample


def emit(tc, ins, outs):
    nc = tc.nc
    XT, WTT, BT = ins["XT"], ins["WTT"], ins["BT"]
    WIH, WHH, ONES = ins["WIH"], ins["WHH"], ins["ONES"]
    DW, DB = ins["DW"], ins["DB"]
    OUT = outs["OUT"]

    with (
        tc.tile_pool(name="consts", bufs=1) as consts,
        tc.tile_pool(name="cpool", bufs=2) as cpool,
        tc.tile_pool(name="sgpool", bufs=2) as sgpool,
        tc.tile_pool(name="small", bufs=2) as small,
        tc.tile_pool(name="hpool", bufs=4) as hpool,
        tc.tile_pool(name="hstp", bufs=2) as hstp,
    ):
        # ---- constants / weights ----
        wtt = consts.tile([NF, TS], BF16)
        nc.sync.dma_start(wtt, WTT)
        bt = consts.tile([TS, 1], F32)
        nc.sync.dma_start(bt, BT)
        wih = consts.tile([TS + 1, 2, G4], BF16)
        whh = consts.tile([H, 2, G4], BF16)
        # HT: attention layout [j, t, 2h] filled by batched DMA transposes
        ht = consts.tile([J, T, 2 * H], BF16)
        SB = 8  # steps per h-staging slab / per batched transpose

        with (
            tc.tile_pool(name="xs2p", bufs=1) as xs2p,
            tc.tile_pool(name="xtp", bufs=2) as xtp,
            tc.tile_pool(name="tfp", bufs=2, space="PSUM") as tfp,
            tc.tile_pool(name="gates", bufs=3, space="PSUM") as gates,
        ):
            # ---- feature transform: xs2[0:64, (t,b,d)] = tanh(Wt@X.T+bt)
            xs2 = xs2p.tile([TS + 1, R], BF16)
            nc.sync.dma_start(xs2[TS : TS + 1, :], ONES)
            xt_tiles = {}

            def feature_round(rr):
                # 512-col round (1 PSUM bank); one big DMA per 4 rounds
                if rr % 4 == 0 and rr // 4 not in xt_tiles:
                    xt = xtp.tile([NF, 2048], BF16, tag="xt", name=f"xt{rr}")
                    nc.sync.dma_start(xt, XT[:, rr * 512 : rr * 512 + 2048])
                    xt_tiles[rr // 4] = xt
                xt = xt_tiles[rr // 4]
                pz = tfp.tile([TS, 512], F32, tag="pz", name=f"pz{rr}")
                nc.tensor.matmul(
                    pz, lhsT=wtt, rhs=xt[:, (rr % 4) * 512 : (rr % 4 + 1) * 512],
                    start=True, stop=True,
                )
                nc.scalar.activation(
                    out=xs2[0:TS, rr * 512 : (rr + 1) * 512],
                    in_=pz, func=AF.Tanh, bias=bt, scale=1.0,
                )


            # rounds 0-5 up front; the rest interleave into early loop steps.
            # round r covers t in [4r, 4r+4) and must be done before the xW
            # prefetch of step 4r-2 touches it.
            feature_round(0)
            nc.sync.dma_start(wih, WIH)
            nc.sync.dma_start(whh, WHH)
            for rr in range(1, 6):
                feature_round(rr)
            feat_sched = {4 * r - 8: r for r in range(6, R // 512)}

            # ---- recurrence ----
            h_prev = [None, None]
            c_prev = [None, None]
            for d in range(2):
                h0 = hpool.tile([H, J], BF16, tag=f"h{d}", name=f"hz{d}")
                nc.vector.memset(h0, 0.0)
                c0 = cpool.tile([H, J], BF16, tag=f"c{d}", name=f"cz{d}")
                nc.vector.memset(c0, 0.0)
                h_prev[d] = h0
                c_prev[d] = c0

            if True:

                def emit_xw(tg):
                    """xW matmuls for step tg (one 2-bank tile, both dirs)."""
                    pg = gates.tile([H, 2, 4, J], F32, tag="g", name=f"pg{tg}")
                    rhs_x = xs2[:, tg * J : (tg + 1) * J]
                    for d in range(2):
                        for c in range(4):
                            nc.tensor.matmul(
                                pg[:, d, c, :],
                                lhsT=wih[:, d, c * H : (c + 1) * H],
                                rhs=rhs_x, start=(c == 0), stop=False,
                            )
                    return pg

                nxt = emit_xw(0)
                hst = None
                for t in range(T):
                    if t % SB == 0:
                        # [h, s, dir, j] staging slab for this 8-step group
                        hst = hstp.tile([H, SB, 2, J], BF16, tag="hst",
                                        name=f"hst{t // SB}")
                    psg = nxt
                    # hW matmuls for this step (both dirs).  stop=True per
                    # gate region: the (i,f,g) sigmoid only gates on its own
                    # three chunks, so it can fire while the o-chunk matmul
                    # is still streaming.
                    for d in range(2):
                        for c in range(4):
                            nc.tensor.matmul(
                                psg[:, d, c, :],
                                lhsT=whh[:, d, c * H : (c + 1) * H],
                                rhs=h_prev[d], start=False,
                                stop=True,
                            )
                    # prefetch next step's xW right behind this step's hW;
                    # remaining feature rounds ride the loop's idle slack
                    if t + 1 < T:
                        nxt = emit_xw(t + 1)
                    if t in feat_sched:
                        feature_round(feat_sched[t])
                    # activations first (Act queue order: sig f, sig b,
                    # fused o-gate sigmoid for both dirs)
                    sg = sgpool.tile([H, 2, 4, J], BF16, tag="sg", name="sg")
                    for d in range(2):
                        nc.scalar.activation(
                            out=sg[:, d, 0:3, :], in_=psg[:, d, 0:3, :],
                            func=AF.Sigmoid,
                        )
                    nc.scalar.activation(
                        out=sg[:, :, 3, :], in_=psg[:, :, 3, :],
                        func=AF.Sigmoid,
                    )
                    # c-chains on DVE
                    cn_ = [None, None]
                    for d in range(2):
                        c2 = small.tile([H, J], BF16, tag=f"c2{d}", name=f"c2{d}")
                        nc.vector.tensor_mul(c2, sg[:, d, 1, :], c_prev[d])
                        u2 = small.tile([H, J], BF16, tag=f"u2{d}", name=f"u2{d}")
                        nc.vector.scalar_tensor_tensor(
                            u2, sg[:, d, 2, :], -0.5, sg[:, d, 0, :],
                            op0=ALU.add, op1=ALU.mult,
                        )
                        cn = cpool.tile([H, J], BF16, tag=f"c{d}", name=f"cn{d}")
                        nc.vector.scalar_tensor_tensor(
                            cn, u2, 2.0, c2, op0=ALU.mult, op1=ALU.add,
                        )
                        cn_[d] = cn
                    # tanh(c) (Act queue: th f, th b), then hn on DVE
                    th_ = [None, None]
                    for d in range(2):
                        th_[d] = small.tile([H, J], BF16, tag=f"th{d}",
                                            name=f"th{d}")
                        nc.scalar.activation(out=th_[d], in_=cn_[d], func=AF.Tanh)
                    for d in range(2):
                        hn = hst[:, t % SB, d, :]
                        nc.vector.tensor_mul(hn, sg[:, d, 3, :], th_[d])
                        h_prev[d] = hn
                        c_prev[d] = cn_[d]
                    # one batched xbar transpose per SB steps: slab
                    # [H, SB*2*J] -> ht[:, t0:t0+SB, :] as [J, SB*2, H]
                    if t % SB == SB - 1:
                        t0 = t - (SB - 1)
                        dst = ht[:, t0 : t0 + SB, :].rearrange(
                            "j s (d h) -> j (s d) h", d=2
                        )
                        nc.sync.dma_start_transpose(dst, hst)

        # ---- tail: attention pooling + context norm ----
        with (
            tc.tile_pool(name="tailp", bufs=1) as tailp,
            tc.tile_pool(name="tailps", bufs=1, space="PSUM") as tailps,
        ):
            htj = ht[:, T - 1, :]  # [J, 2H] last hidden state
            htj_b = bass.AP(
                tensor=htj.tensor, offset=htj.offset,
                ap=[list(htj.ap[0]), [0, T], list(htj.ap[-1])],
            )
            prod = tailp.tile([J, T, 2 * H], BF16)
            nc.vector.tensor_mul(prod, ht, htj_b)
            # pairwise-tree sum over h: bf16 levels ping-pong {pp0, prod}
            pp0 = tailp.tile([J, T, 128], BF16)
            nc.vector.tensor_add(pp0, prod[:, :, 0:128], prod[:, :, 128:256])
            nc.vector.tensor_add(prod[:, :, 0:64], pp0[:, :, 0:64], pp0[:, :, 64:128])
            nc.vector.tensor_add(pp0[:, :, 0:32], prod[:, :, 0:32], prod[:, :, 32:64])
            nc.vector.tensor_add(prod[:, :, 0:16], pp0[:, :, 0:16], pp0[:, :, 16:32])
            nc.vector.tensor_add(pp0[:, :, 0:8], prod[:, :, 0:8], prod[:, :, 8:16])
            ltrf = tailp.tile([J, T, 4], F32)
            nc.vector.tensor_add(ltrf, pp0[:, :, 0:4], pp0[:, :, 4:8])
            w = 4
            while w > 1:
                w //= 2
                nc.vector.tensor_add(ltrf[:, :, 0:w], ltrf[:, :, 0:w],
                                     ltrf[:, :, w : 2 * w])
            logits = ltrf[:, :, 0:1].rearrange("j t one -> j (t one)")
            mx = tailp.tile([J, 1], F32)
            nc.vector.tensor_reduce(mx, logits, axis=mybir.AxisListType.X, op=ALU.max)
            mxn = tailp.tile([J, 1], F32)
            nc.vector.tensor_scalar_mul(mxn, mx, -1.0)
            ew = tailp.tile([J, T], F32)
            dsum = tailp.tile([J, 1], F32)
            nc.scalar.activation(out=ew, in_=logits, func=AF.Exp, bias=mxn,
                                 scale=1.0, accum_out=dsum)
            rd = tailp.tile([J, 1], F32)
            nc.vector.reciprocal(rd, dsum)
            nc.vector.tensor_scalar_mul(ew, ew, rd)  # softmax weights in place
            prod2 = tailp.tile([J, T, 2 * H], BF16, tag="prod")  # reuse slab
            # ew[j,t] is a per-partition scalar for fixed t.  First 64 t's:
            # plain weighted copies (split DVE/Scalar); second 64 t's fold
            # the tree's first level in: prod2[p] += ew[p+64]*ht[p+64].
            for tt in range(64):
                if tt % 5 == 4:
                    nc.vector.tensor_scalar_mul(prod2[:, tt, :], ht[:, tt, :],
                                                ew[:, tt : tt + 1])
                else:
                    nc.scalar.activation(out=prod2[:, tt, :], in_=ht[:, tt, :],
                                         func=AF.Copy, scale=ew[:, tt : tt + 1])
            for tt in range(64, T):
                nc.vector.scalar_tensor_tensor(
                    prod2[:, tt - 64, :], ht[:, tt, :], ew[:, tt : tt + 1],
                    prod2[:, tt - 64, :], op0=ALU.mult, op1=ALU.add,
                )
            # preload the sqrt table set while DVE runs the tree below; the
            # Copy ops above ran from the exp set, and everything after this
            # point (Square, Sqrt) lives in the sqrt set too.
            sqrt_warm = tailp.tile([1, 1], F32)
            nc.scalar.activation(out=sqrt_warm, in_=mx[0:1, :], func=AF.Sqrt)
            # pairwise-tree sum over t (prod2[0:64] holds pair sums already)
            qq = pp0.rearrange("j a b -> j (a b)").rearrange(
                "j (a b) -> j a b", a=64)
            nc.vector.tensor_add(qq[:, 0:32, :], prod2[:, 0:32, :],
                                 prod2[:, 32:64, :])
            nc.vector.tensor_add(prod2[:, 0:16, :], qq[:, 0:16, :],
                                 qq[:, 16:32, :])
            nc.vector.tensor_add(qq[:, 0:8, :], prod2[:, 0:8, :],
                                 prod2[:, 8:16, :])
            nc.vector.tensor_add(prod2[:, 0:4, :], qq[:, 0:4, :], qq[:, 4:8, :])
            ptrf = tailp.tile([J, 2, 2 * H], F32)
            nc.vector.tensor_add(ptrf, prod2[:, 0:2, :], prod2[:, 2:4, :])
            nc.vector.tensor_add(ptrf[:, 0:1, :], ptrf[:, 0:1, :],
                                 ptrf[:, 1:2, :])
            pooled = ptrf[:, 0:1, :].rearrange("j one p -> j (one p)")

            # context norm across each sample's (d, 2h) block
            pooled2 = tailp.tile([J, 2 * H], F32)
            nc.vector.tensor_mul(pooled2, pooled, pooled)
            sel = tailp.tile([J, BLOC], F32)
            nc.sync.dma_start(sel, ins["SEL"])
            pstat = tailps.tile([BLOC, 2 * G4], F32, tag="stats")
            nc.tensor.matmul(pstat[:, 0 : 2 * H], lhsT=sel, rhs=pooled,
                             start=True, stop=False)
            nc.tensor.matmul(pstat[:, 2 * H : 4 * H], lhsT=sel, rhs=pooled2,
                             start=False, stop=True)
            s1 = tailp.tile([BLOC, 1], F32)
            nc.vector.tensor_reduce(s1, pstat[:, 0 : 2 * H],
                                    axis=mybir.AxisListType.X, op=ALU.add)
            s2 = tailp.tile([BLOC, 1], F32)
            nc.vector.tensor_reduce(s2, pstat[:, 2 * H : 4 * H],
                                    axis=mybir.AxisListType.X, op=ALU.add)
            stats2 = tailp.tile([BLOC, 2], F32)
            nc.scalar.mul(stats2[:, 0:1], s1, 1.0 / NORM_N)      # mean
            q = tailp.tile([BLOC, 1], F32)
            nc.vector.tensor_mul(q, s1, stats2[:, 0:1])          # sum*mean
            v = tailp.tile([BLOC, 1], F32)
            nc.vector.tensor_tensor(v, s2, q, op=ALU.subtract)
            sd = tailp.tile([BLOC, 1], F32)
            nc.scalar.activation(out=sd, in_=v, func=AF.Sqrt,
                                 scale=1.0 / (NORM_N - 1))
            nc.vector.reciprocal(stats2[:, 1:2], sd)             # rstd
            selt = tailp.tile([BLOC, J], F32)
            nc.sync.dma_start(selt, ins["SELT"])
            pmb = tailps.tile([J, 2], F32, tag="mb")
            nc.tensor.matmul(pmb, lhsT=selt, rhs=stats2, start=True, stop=True)
            mb = tailp.tile([J, 2], F32)
            nc.vector.tensor_copy(mb, pmb)
            dwt = tailp.tile([J, 2 * H], F32)
            nc.sync.dma_start(dwt[0:D, :], DW)
            nc.sync.dma_start(dwt[D:J, :], DW)
            dbt = tailp.tile([J, 2 * H], F32)
            nc.sync.dma_start(dbt[0:D, :], DB)
            nc.sync.dma_start(dbt[D:J, :], DB)
            t1 = tailp.tile([J, 2 * H], F32)
            nc.vector.tensor_scalar(t1, pooled, mb[:, 0:1], mb[:, 1:2],
                                    op0=ALU.subtract, op1=ALU.mult)
            t2 = tailp.tile([J, 2 * H], F32)
            nc.vector.tensor_mul(t2, t1, dwt)
            t3 = tailp.tile([J, 2 * H], F32)
            nc.vector.tensor_add(t3, t2, dbt)
            nc.sync.dma_start(OUT, t3)


def build_program():
    nc = bacc.Bacc("TRN2", target_bir_lowering=False, debug=False)
    ins = {
        "XT": nc.dram_tensor("XT", [NF, R], BF16, kind="ExternalInput").ap(),
        "WTT": nc.dram_tensor("WTT", [NF, TS], BF16, kind="ExternalInput").ap(),
        "BT": nc.dram_tensor("BT", [TS, 1], F32, kind="ExternalInput").ap(),
        "WIH": nc.dram_tensor("WIH", [TS + 1, 2, G4], BF16, kind="ExternalInput").ap(),
        "WHH": nc.dram_tensor("WHH", [H, 2, G4], BF16, kind="ExternalInput").ap(),
        "ONES": nc.dram_tensor("ONES", [1, R], BF16, kind="ExternalInput").ap(),
        "DW": nc.dram_tensor("DW", [D, 2 * H], F32, kind="ExternalInput").ap(),
        "SEL": nc.dram_tensor("SEL", [J, BLOC], F32, kind="ExternalInput").ap(),
        "SELT": nc.dram_tensor("SELT", [BLOC, J], F32, kind="ExternalInput").ap(),
        "DB": nc.dram_tensor("DB", [D, 2 * H], F32, kind="ExternalInput").ap(),
    }
    outs = {
        "OUT": nc.dram_tensor("OUT", [J, 2 * H], F32, kind="ExternalOutput").ap(),
    }
    with tile.TileContext(nc) as tc:
        emit(tc, ins, outs)
    nc.compile()
    return nc


def _prep_dir(Wih, Whh, bih, bhh):
    # gate order (i,f,o,g); the g block is pre-scaled by 2 so the kernel can
    # evaluate tanh(g) as 2*sigmoid(2g)-1 inside the fused sigmoid op
    wihT = Wih.T.reshape(TS, 4, H)[:, PERM, :].reshape(TS, G4).copy()
    biasr = (bih + bhh).reshape(4, H)[PERM, :].reshape(G4).copy()
    wihT[:, 2 * H : 3 * H] *= 2.0
    biasr[2 * H : 3 * H] *= 2.0
    wih65 = np.concatenate([wihT, biasr[None, :]], axis=0).astype(BF16NP)
    whhT = Whh.T.reshape(H, 4, H)[:, PERM, :].reshape(H, G4).copy()
    whhT[:, 2 * H : 3 * H] *= 2.0
    whhT = whhT.astype(BF16NP)
    return wih65, whhT


def prep_inputs(X, W_t, b_t, Wih_f, Whh_f, bih_f, bhh_f,
                Wih_b, Whh_b, bih_b, bhh_b, diag_w, diag_b):
    wih_f, whh_f = _prep_dir(Wih_f, Whh_f, bih_f, bhh_f)
    wih_b, whh_b = _prep_dir(Wih_b, Whh_b, bih_b, bhh_b)
    shared = {
        "WTT": np.ascontiguousarray(W_t.T, dtype=BF16NP),
        "BT": np.ascontiguousarray(b_t.reshape(TS, 1), dtype=np.float32),
        "WIH": np.ascontiguousarray(np.stack([wih_f, wih_b], axis=1)),
        "WHH": np.ascontiguousarray(np.stack([whh_f, whh_b], axis=1)),
        "ONES": np.ones((1, R), dtype=BF16NP),
        "SEL": np.kron(np.eye(BLOC, dtype=np.float32), np.ones((D, 1), np.float32)),
        "SELT": np.kron(np.eye(BLOC, dtype=np.float32), np.ones((1, D), np.float32)),
        "DW": np.ascontiguousarray(diag_w.reshape(D, 2 * H), dtype=np.float32),
        "DB": np.ascontiguousarray(diag_b.reshape(D, 2 * H), dtype=np.float32),
    }
    in_maps = []
    for i in range(NCORES):
        xt = np.ascontiguousarray(
            X[i * BLOC : (i + 1) * BLOC].transpose(3, 1, 0, 2).reshape(NF, R),
            dtype=BF16NP,
        )
        m = {"XT": xt}
        m.update(shared)
        in_maps.append(m)
    return in_maps


def kernel(**inputs):
    inputs = {k: np.asarray(v, dtype=np.float32) for k, v in inputs.items()}
    in_maps = prep_inputs(**inputs)
    nc = build_program()
    res = run_bass_kernel_spmd(nc, in_maps, list(range(NCORES)))
    out = np.concatenate(
        [res.results[i]["OUT"].reshape(BLOC, D, 2 * H) for i in range(NCORES)],
        axis=0,
    )
    return np.ascontiguousarray(out, dtype=np.float32)


if __name__ == "__main__":
    nc = build_program()
    print("program built ok")



# revision 22
# speedup vs baseline: 1.2328x; 1.2328x over previous
"""Trainium2 Bass kernel for nn_ContextEncoder, v2.

Structure per core (J=128 sequences = 2 samples x 64 d):
  - feature: xs2[0:64, (t,b,d)] = tanh(Wt @ X.T + bt), bf16 matmuls,
    4-bank PSUM rounds with one FD=2048 tanh per round.
  - recurrence: per step, per dir: 4 hW matmuls N=128 into a 2-step PSUM
    group (double-buffered, xW for the next group prefetched between
    steps), one sigmoid FD=512, DVE chain of 4 ops using
    scalar_tensor_tensor fusions, one tanh FD=128.
    g-gate weights pre-scaled by 2 so tanh(g) = 2*sig(2g)-1; the cell is
      u2 = (sig2g - 0.5) * sig_i          (STT)
      c2 = sig_f * c_prev                 (TT)
      cn = 2*u2 + c2                      (STT)
      hn = sig_o * tanh(cn)               (TT)
  - h history goes to HT[j, t, 2h] via DMA xbar transposes (sync queue
    for dir 0, scalar queue for dir 1); no PE transposes, no PSUM slot.
  - tail: attention pooling (prod + pairwise trees + softmax + per-t
    weighted sum) and context norm via SEL matmuls.
"""

import sys

for _p in ("/opt/trn_rl_repo", "/root/.axon_site/_ro/trn_rl_repo"):
    if _p not in sys.path:
        sys.path.append(_p)

import numpy as np
import ml_dtypes

import concourse.bass as bass
import concourse.bacc as bacc
import concourse.tile as tile
from concourse import mybir
from concourse.bass_utils import run_bass_kernel_spmd

BF16NP = np.float16
F32 = mybir.dt.float32
BF16 = mybir.dt.float16
AF = mybir.ActivationFunctionType
ALU = mybir.AluOpType

B, T, D, NF = 16, 128, 64, 32
TS, H = 64, 128
NCORES = 8
BLOC = B // NCORES          # 2 samples per core
J = BLOC * D                # 128 sequences per core
R = J * T                   # 16384 (t, b, d) columns
G4 = 4 * H                  # 512 gates per direction
PERM = (0, 1, 2, 3)         # torch gate order kept as (i,f,g,o)
NORM_N = D * 2 * H          # 16384 context-norm elements per sample


def emit(tc, ins, outs):
    nc = tc.nc
    XT, WTT, BT = ins["XT"], ins["WTT"], ins["BT"]
    WIH, WHH, ONES = ins["WIH"], ins["WHH"], ins["ONES"]
    DW, DB = ins["DW"], ins["DB"]
    OUT = outs["OUT"]

    with (
        tc.tile_pool(name="consts", bufs=1) as consts,
        tc.tile_pool(name="cpool", bufs=2) as cpool,
        tc.tile_pool(name="sgpool", bufs=2) as sgpool,
        tc.tile_pool(name="small", bufs=2) as small,
        tc.tile_pool(name="hpool", bufs=4) as hpool,
        tc.tile_pool(name="hstp", bufs=2) as hstp,
    ):
        # ---- constants / weights ----
        wtt = consts.tile([NF, TS], BF16)
        nc.sync.dma_start(wtt, WTT)
        bt = consts.tile([TS, 1], F32)
        nc.sync.dma_start(bt, BT)
        wih = consts.tile([TS + 1, 2, G4], BF16)
        whh = consts.tile([H, 2, G4], BF16)
        # HT: attention layout [j, t, 2h] filled by batched DMA transposes
        ht = consts.tile([J, T, 2 * H], BF16)
        SB = 8  # steps per h-staging slab / per batched transpose

        with (
            tc.tile_pool(name="xs2p", bufs=1) as xs2p,
            tc.tile_pool(name="xtp", bufs=2) as xtp,
            tc.tile_pool(name="tfp", bufs=2, space="PSUM") as tfp,
            tc.tile_pool(name="gates", bufs=3, space="PSUM") as gates,
        ):
            # ---- feature transform: xs2[0:64, (t,b,d)] = tanh(Wt@X.T+bt)
            xs2 = xs2p.tile([TS + 1, R], BF16)
            nc.sync.dma_start(xs2[TS : TS + 1, :], ONES)
            xt_tiles = {}

            def feature_round(rr):
                # 512-col round (1 PSUM bank); one big DMA per 4 rounds
                if rr % 4 == 0 and rr // 4 not in xt_tiles:
                    xt = xtp.tile([NF, 2048], BF16, tag="xt", name=f"xt{rr}")
                    nc.sync.dma_start(xt, XT[:, rr * 512 : rr * 512 + 2048])
                    xt_tiles[rr // 4] = xt
                xt = xt_tiles[rr // 4]
                pz = tfp.tile([TS, 512], F32, tag="pz", name=f"pz{rr}")
                nc.tensor.matmul(
                    pz, lhsT=wtt, rhs=xt[:, (rr % 4) * 512 : (rr % 4 + 1) * 512],
                    start=True, stop=True,
                )
                nc.scalar.activation(
                    out=xs2[0:TS, rr * 512 : (rr + 1) * 512],
                    in_=pz, func=AF.Tanh, bias=bt, scale=1.0,
                )


            # rounds 0-5 up front; the rest interleave into early loop steps.
            # round r covers t in [4r, 4r+4) and must be done before the xW
            # prefetch of step 4r-2 touches it.
            feature_round(0)
            nc.sync.dma_start(wih, WIH)
            nc.sync.dma_start(whh, WHH)
            for rr in range(1, 6):
                feature_round(rr)
            feat_sched = {4 * r - 8: r for r in range(6, R // 512)}

            # ---- recurrence ----
            h_prev = [None, None]
            c_prev = [None, None]
            for d in range(2):
                h0 = hpool.tile([H, J], BF16, tag=f"h{d}", name=f"hz{d}")
                nc.vector.memset(h0, 0.0)
                c0 = cpool.tile([H, J], BF16, tag=f"c{d}", name=f"cz{d}")
                nc.vector.memset(c0, 0.0)
                h_prev[d] = h0
                c_prev[d] = c0

            if True:

                def emit_xw(tg):
                    """xW matmuls for step tg (one 1-bank tile per dir)."""
                    tiles = [None, None]
                    for d in range(2):
                        pg = gates.tile([H, 4, J], F32, tag=f"g{d}",
                                        name=f"pg{d}_{tg}")
                        tiles[d] = pg
                        rhs_x = xs2[:, tg * J : (tg + 1) * J]
                        for c in range(4):
                            nc.tensor.matmul(
                                pg[:, c, :],
                                lhsT=wih[:, d, c * H : (c + 1) * H],
                                rhs=rhs_x, start=(c == 0), stop=False,
                            )
                    return tiles

                nxt = emit_xw(0)
                hst = None
                for t in range(T):
                    if t % SB == 0:
                        # [h, s, dir, j] staging slab for this 8-step group
                        hst = hstp.tile([H, SB, 2, J], BF16, tag="hst",
                                        name=f"hst{t // SB}")
                    psg = nxt
                    # hW matmuls for this step (both dirs)
                    for d in range(2):
                        for c in range(4):
                            nc.tensor.matmul(
                                psg[d][:, c, :],
                                lhsT=whh[:, d, c * H : (c + 1) * H],
                                rhs=h_prev[d], start=False,
                                stop=(c == 3),
                            )
                    # prefetch next step's xW right behind this step's hW;
                    # remaining feature rounds ride the loop's idle slack
                    if t + 1 < T:
                        nxt = emit_xw(t + 1)
                    if t in feat_sched:
                        feature_round(feat_sched[t])
                    # activations first (Act queue order: sig f, sig b)
                    sg = [None, None]
                    for d in range(2):
                        sg[d] = sgpool.tile([H, 4, J], BF16, tag=f"sg{d}",
                                            name=f"sg{d}")
                        nc.scalar.activation(
                            out=sg[d][:, 0:3, :], in_=psg[d][:, 0:3, :],
                            func=AF.Sigmoid,
                        )
                    for d in range(2):
                        nc.scalar.activation(
                            out=sg[d][:, 3, :], in_=psg[d][:, 3, :],
                            func=AF.Sigmoid,
                        )
                    # c-chains on DVE
                    # cell state tracked as S = c/2, so the update is a plain
                    # 2x-mode tensor add and tanh(c) = tanh(2S) rides the
                    # activation's free input scale.
                    cn_ = [None, None]
                    for d in range(2):
                        c2 = small.tile([H, J], BF16, tag=f"c2{d}", name=f"c2{d}")
                        nc.vector.tensor_mul(c2, sg[d][:, 1, :], c_prev[d])
                        u2 = small.tile([H, J], BF16, tag=f"u2{d}", name=f"u2{d}")
                        nc.vector.scalar_tensor_tensor(
                            u2, sg[d][:, 2, :], -0.5, sg[d][:, 0, :],
                            op0=ALU.add, op1=ALU.mult,
                        )
                        cn = cpool.tile([H, J], BF16, tag=f"c{d}", name=f"cn{d}")
                        nc.vector.tensor_add(cn, u2, c2)
                        cn_[d] = cn
                    # tanh(2S) (Act queue: th f, th b), then hn on DVE
                    th_ = [None, None]
                    for d in range(2):
                        th_[d] = small.tile([H, J], BF16, tag=f"th{d}",
                                            name=f"th{d}")
                        nc.scalar.activation(out=th_[d], in_=cn_[d],
                                             func=AF.Tanh, scale=2.0)
                    for d in range(2):
                        hn = hst[:, t % SB, d, :]
                        nc.vector.tensor_mul(hn, sg[d][:, 3, :], th_[d])
                        h_prev[d] = hn
                        c_prev[d] = cn_[d]
                    # one batched xbar transpose per SB steps: slab
                    # [H, SB*2*J] -> ht[:, t0:t0+SB, :] as [J, SB*2, H]
                    if t % SB == SB - 1:
                        t0 = t - (SB - 1)
                        dst = ht[:, t0 : t0 + SB, :].rearrange(
                            "j s (d h) -> j (s d) h", d=2
                        )
                        nc.sync.dma_start_transpose(dst, hst)

        # ---- tail: attention pooling + context norm ----
        with (
            tc.tile_pool(name="tailp", bufs=1) as tailp,
            tc.tile_pool(name="tailps", bufs=1, space="PSUM") as tailps,
        ):
            htj = ht[:, T - 1, :]  # [J, 2H] last hidden state
            htj_b = bass.AP(
                tensor=htj.tensor, offset=htj.offset,
                ap=[list(htj.ap[0]), [0, T], list(htj.ap[-1])],
            )
            prod = tailp.tile([J, T, 2 * H], BF16)
            nc.vector.tensor_mul(prod, ht, htj_b)
            # pairwise-tree sum over h: bf16 levels ping-pong {pp0, prod}
            pp0 = tailp.tile([J, T, 128], BF16)
            nc.vector.tensor_add(pp0, prod[:, :, 0:128], prod[:, :, 128:256])
            nc.vector.tensor_add(prod[:, :, 0:64], pp0[:, :, 0:64], pp0[:, :, 64:128])
            nc.vector.tensor_add(pp0[:, :, 0:32], prod[:, :, 0:32], prod[:, :, 32:64])
            nc.vector.tensor_add(prod[:, :, 0:16], pp0[:, :, 0:16], pp0[:, :, 16:32])
            nc.vector.tensor_add(pp0[:, :, 0:8], prod[:, :, 0:8], prod[:, :, 8:16])
            ltrf = tailp.tile([J, T, 4], F32)
            nc.vector.tensor_add(ltrf, pp0[:, :, 0:4], pp0[:, :, 4:8])
            w = 4
            while w > 1:
                w //= 2
                nc.vector.tensor_add(ltrf[:, :, 0:w], ltrf[:, :, 0:w],
                                     ltrf[:, :, w : 2 * w])
            logits = ltrf[:, :, 0:1].rearrange("j t one -> j (t one)")
            mx = tailp.tile([J, 1], F32)
            nc.vector.tensor_reduce(mx, logits, axis=mybir.AxisListType.X, op=ALU.max)
            mxn = tailp.tile([J, 1], F32)
            nc.vector.tensor_scalar_mul(mxn, mx, -1.0)
            ew = tailp.tile([J, T], F32)
            dsum = tailp.tile([J, 1], F32)
            nc.scalar.activation(out=ew, in_=logits, func=AF.Exp, bias=mxn,
                                 scale=1.0, accum_out=dsum)
            rd = tailp.tile([J, 1], F32)
            nc.vector.reciprocal(rd, dsum)
            nc.vector.tensor_scalar_mul(ew, ew, rd)  # softmax weights in place
            prod2 = tailp.tile([J, T, 2 * H], BF16, tag="prod")  # reuse slab
            # ew[j,t] is a per-partition scalar for fixed t.  First 64 t's:
            # plain weighted copies (split DVE/Scalar); second 64 t's fold
            # the tree's first level in: prod2[p] += ew[p+64]*ht[p+64].
            for tt in range(64):
                if tt % 5 == 4:
                    nc.vector.tensor_scalar_mul(prod2[:, tt, :], ht[:, tt, :],
                                                ew[:, tt : tt + 1])
                else:
                    nc.scalar.activation(out=prod2[:, tt, :], in_=ht[:, tt, :],
                                         func=AF.Copy, scale=ew[:, tt : tt + 1])
            for tt in range(64, T):
                nc.vector.scalar_tensor_tensor(
                    prod2[:, tt - 64, :], ht[:, tt, :], ew[:, tt : tt + 1],
                    prod2[:, tt - 64, :], op0=ALU.mult, op1=ALU.add,
                )
            # preload the sqrt table set while DVE runs the tree below; the
            # Copy ops above ran from the exp set, and everything after this
            # point (Square, Sqrt) lives in the sqrt set too.
            sqrt_warm = tailp.tile([1, 1], F32)
            nc.scalar.activation(out=sqrt_warm, in_=mx[0:1, :], func=AF.Sqrt)
            # pairwise-tree sum over t (prod2[0:64] holds pair sums already)
            qq = pp0.rearrange("j a b -> j (a b)").rearrange(
                "j (a b) -> j a b", a=64)
            nc.vector.tensor_add(qq[:, 0:32, :], prod2[:, 0:32, :],
                                 prod2[:, 32:64, :])
            nc.vector.tensor_add(prod2[:, 0:16, :], qq[:, 0:16, :],
                                 qq[:, 16:32, :])
            nc.vector.tensor_add(qq[:, 0:8, :], prod2[:, 0:8, :],
                                 prod2[:, 8:16, :])
            nc.vector.tensor_add(prod2[:, 0:4, :], qq[:, 0:4, :], qq[:, 4:8, :])
            ptrf = tailp.tile([J, 2, 2 * H], F32)
            nc.vector.tensor_add(ptrf, prod2[:, 0:2, :], prod2[:, 2:4, :])
            nc.vector.tensor_add(ptrf[:, 0:1, :], ptrf[:, 0:1, :],
                                 ptrf[:, 1:2, :])
            pooled = ptrf[:, 0:1, :].rearrange("j one p -> j (one p)")

            # context norm across each sample's (d, 2h) block
            pooled2 = tailp.tile([J, 2 * H], F32)
            nc.vector.tensor_mul(pooled2, pooled, pooled)
            sel = tailp.tile([J, BLOC], F32)
            nc.sync.dma_start(sel, ins["SEL"])
            pstat = tailps.tile([BLOC, 2 * G4], F32, tag="stats")
            nc.tensor.matmul(pstat[:, 0 : 2 * H], lhsT=sel, rhs=pooled,
                             start=True, stop=False)
            nc.tensor.matmul(pstat[:, 2 * H : 4 * H], lhsT=sel, rhs=pooled2,
                             start=False, stop=True)
            s1 = tailp.tile([BLOC, 1], F32)
            nc.vector.tensor_reduce(s1, pstat[:, 0 : 2 * H],
                                    axis=mybir.AxisListType.X, op=ALU.add)
            s2 = tailp.tile([BLOC, 1], F32)
            nc.vector.tensor_reduce(s2, pstat[:, 2 * H : 4 * H],
                                    axis=mybir.AxisListType.X, op=ALU.add)
            stats2 = tailp.tile([BLOC, 2], F32)
            nc.scalar.mul(stats2[:, 0:1], s1, 1.0 / NORM_N)      # mean
            q = tailp.tile([BLOC, 1], F32)
            nc.vector.tensor_mul(q, s1, stats2[:, 0:1])          # sum*mean
            v = tailp.tile([BLOC, 1], F32)
            nc.vector.tensor_tensor(v, s2, q, op=ALU.subtract)
            sd = tailp.tile([BLOC, 1], F32)
            nc.scalar.activation(out=sd, in_=v, func=AF.Sqrt,
                                 scale=1.0 / (NORM_N - 1))
            nc.vector.reciprocal(stats2[:, 1:2], sd)             # rstd
            selt = tailp.tile([BLOC, J], F32)
            nc.sync.dma_start(selt, ins["SELT"])
            pmb = tailps.tile([J, 2], F32, tag="mb")
            nc.tensor.matmul(pmb, lhsT=selt, rhs=stats2, start=True, stop=True)
            mb = tailp.tile([J, 2], F32)
            nc.vector.tensor_copy(mb, pmb)
            dwt = tailp.tile([J, 2 * H], F32)
            nc.sync.dma_start(dwt[0:D, :], DW)
            nc.sync.dma_start(dwt[D:J, :], DW)
            dbt = tailp.tile([J, 2 * H], F32)
            nc.sync.dma_start(dbt[0:D, :], DB)
            nc.sync.dma_start(dbt[D:J, :], DB)
            t1 = tailp.tile([J, 2 * H], F32)
            nc.vector.tensor_scalar(t1, pooled, mb[:, 0:1], mb[:, 1:2],
                                    op0=ALU.subtract, op1=ALU.mult)
            t2 = tailp.tile([J, 2 * H], F32)
            nc.vector.tensor_mul(t2, t1, dwt)
            t3 = tailp.tile([J, 2 * H], F32)
            nc.vector.tensor_add(t3, t2, dbt)
            nc.sync.dma_start(OUT, t3)


def build_program():
    nc = bacc.Bacc("TRN2", target_bir_lowering=False, debug=False)
    ins = {
        "XT": nc.dram_tensor("XT", [NF, R], BF16, kind="ExternalInput").ap(),
        "WTT": nc.dram_tensor("WTT", [NF, TS], BF16, kind="ExternalInput").ap(),
        "BT": nc.dram_tensor("BT", [TS, 1], F32, kind="ExternalInput").ap(),
        "WIH": nc.dram_tensor("WIH", [TS + 1, 2, G4], BF16, kind="ExternalInput").ap(),
        "WHH": nc.dram_tensor("WHH", [H, 2, G4], BF16, kind="ExternalInput").ap(),
        "ONES": nc.dram_tensor("ONES", [1, R], BF16, kind="ExternalInput").ap(),
        "DW": nc.dram_tensor("DW", [D, 2 * H], F32, kind="ExternalInput").ap(),
        "SEL": nc.dram_tensor("SEL", [J, BLOC], F32, kind="ExternalInput").ap(),
        "SELT": nc.dram_tensor("SELT", [BLOC, J], F32, kind="ExternalInput").ap(),
        "DB": nc.dram_tensor("DB", [D, 2 * H], F32, kind="ExternalInput").ap(),
    }
    outs = {
        "OUT": nc.dram_tensor("OUT", [J, 2 * H], F32, kind="ExternalOutput").ap(),
    }
    with tile.TileContext(nc) as tc:
        emit(tc, ins, outs)
    nc.compile()
    return nc


def _prep_dir(Wih, Whh, bih, bhh):
    # gate order (i,f,o,g); the g block is pre-scaled by 2 so the kernel can
    # evaluate tanh(g) as 2*sigmoid(2g)-1 inside the fused sigmoid op
    wihT = Wih.T.reshape(TS, 4, H)[:, PERM, :].reshape(TS, G4).copy()
    biasr = (bih + bhh).reshape(4, H)[PERM, :].reshape(G4).copy()
    wihT[:, 2 * H : 3 * H] *= 2.0
    biasr[2 * H : 3 * H] *= 2.0
    wih65 = np.concatenate([wihT, biasr[None, :]], axis=0).astype(BF16NP)
    whhT = Whh.T.reshape(H, 4, H)[:, PERM, :].reshape(H, G4).copy()
    whhT[:, 2 * H : 3 * H] *= 2.0
    whhT = whhT.astype(BF16NP)
    return wih65, whhT


def prep_inputs(X, W_t, b_t, Wih_f, Whh_f, bih_f, bhh_f,
                Wih_b, Whh_b, bih_b, bhh_b, diag_w, diag_b):
    wih_f, whh_f = _prep_dir(Wih_f, Whh_f, bih_f, bhh_f)
    wih_b, whh_b = _prep_dir(Wih_b, Whh_b, bih_b, bhh_b)
    shared = {
        "WTT": np.ascontiguousarray(W_t.T, dtype=BF16NP),
        "BT": np.ascontiguousarray(b_t.reshape(TS, 1), dtype=np.float32),
        "WIH": np.ascontiguousarray(np.stack([wih_f, wih_b], axis=1)),
        "WHH": np.ascontiguousarray(np.stack([whh_f, whh_b], axis=1)),
        "ONES": np.ones((1, R), dtype=BF16NP),
        "SEL": np.kron(np.eye(BLOC, dtype=np.float32), np.ones((D, 1), np.float32)),
        "SELT": np.kron(np.eye(BLOC, dtype=np.float32), np.ones((1, D), np.float32)),
        "DW": np.ascontiguousarray(diag_w.reshape(D, 2 * H), dtype=np.float32),
        "DB": np.ascontiguousarray(diag_b.reshape(D, 2 * H), dtype=np.float32),
    }
    in_maps = []
    for i in range(NCORES):
        xt = np.ascontiguousarray(
            X[i * BLOC : (i + 1) * BLOC].transpose(3, 1, 0, 2).reshape(NF, R),
            dtype=BF16NP,
        )
        m = {"XT": xt}
        m.update(shared)
        in_maps.append(m)
    return in_maps


def kernel(**inputs):
    inputs = {k: np.asarray(v, dtype=np.float32) for k, v in inputs.items()}
    in_maps = prep_inputs(**inputs)
    nc = build_program()
    res = run_bass_kernel_spmd(nc, in_maps, list(range(NCORES)))
    out = np.concatenate(
        [res.results[i]["OUT"].reshape(BLOC, D, 2 * H) for i in range(NCORES)],
        axis=0,
    )
    return np.ascontiguousarray(out, dtype=np.float32)


if __name__ == "__main__":
    nc = build_program()
    print("program built ok")



# revision 28
# speedup vs baseline: 1.2407x; 1.0064x over previous
"""Trainium2 Bass kernel for nn_ContextEncoder, v2.

Structure per core (J=128 sequences = 2 samples x 64 d):
  - feature: xs2[0:64, (t,b,d)] = tanh(Wt @ X.T + bt), bf16 matmuls,
    4-bank PSUM rounds with one FD=2048 tanh per round.
  - recurrence: per step, per dir: 4 hW matmuls N=128 into a 2-step PSUM
    group (double-buffered, xW for the next group prefetched between
    steps), one sigmoid FD=512, DVE chain of 4 ops using
    scalar_tensor_tensor fusions, one tanh FD=128.
    g-gate weights pre-scaled by 2 so tanh(g) = 2*sig(2g)-1; the cell is
      u2 = (sig2g - 0.5) * sig_i          (STT)
      c2 = sig_f * c_prev                 (TT)
      cn = 2*u2 + c2                      (STT)
      hn = sig_o * tanh(cn)               (TT)
  - h history goes to HT[j, t, 2h] via DMA xbar transposes (sync queue
    for dir 0, scalar queue for dir 1); no PE transposes, no PSUM slot.
  - tail: attention pooling (prod + pairwise trees + softmax + per-t
    weighted sum) and context norm via SEL matmuls.
"""

import sys

for _p in ("/opt/trn_rl_repo", "/root/.axon_site/_ro/trn_rl_repo"):
    if _p not in sys.path:
        sys.path.append(_p)

import numpy as np
import ml_dtypes

import concourse.bass as bass
import concourse.bacc as bacc
import concourse.tile as tile
from concourse import mybir
from concourse.bass_utils import run_bass_kernel_spmd

BF16NP = np.float16
F32 = mybir.dt.float32
BF16 = mybir.dt.float16
AF = mybir.ActivationFunctionType
ALU = mybir.AluOpType

B, T, D, NF = 16, 128, 64, 32
TS, H = 64, 128
NCORES = 8
BLOC = B // NCORES          # 2 samples per core
J = BLOC * D                # 128 sequences per core
R = J * T                   # 16384 (t, b, d) columns
G4 = 4 * H                  # 512 gates per direction
PERM = (0, 1, 2, 3)         # torch gate order kept as (i,f,g,o)
NORM_N = D * 2 * H          # 16384 context-norm elements per sample


def emit(tc, ins, outs):
    nc = tc.nc
    XT, WTT, BT = ins["XT"], ins["WTT"], ins["BT"]
    WIH, WHH, ONES = ins["WIH"], ins["WHH"], ins["ONES"]
    DW, DB = ins["DW"], ins["DB"]
    OUT = outs["OUT"]

    with (
        tc.tile_pool(name="consts", bufs=1) as consts,
        tc.tile_pool(name="cpool", bufs=2) as cpool,
        tc.tile_pool(name="sgpool", bufs=2) as sgpool,
        tc.tile_pool(name="small", bufs=2) as small,
        tc.tile_pool(name="hpool", bufs=4) as hpool,
        tc.tile_pool(name="hstp", bufs=2) as hstp,
    ):
        # ---- constants / weights ----
        wtt = consts.tile([NF, TS], BF16)
        nc.sync.dma_start(wtt, WTT)
        bt = consts.tile([TS, 1], F32)
        nc.sync.dma_start(bt, BT)
        wih = consts.tile([TS + 1, 2, G4], BF16)
        whh = consts.tile([H, 2, G4], BF16)
        # HT: attention layout [j, t, 2h] filled by batched DMA transposes
        ht = consts.tile([J, T, 2 * H], BF16)
        SB = 8  # steps per h-staging slab / per batched transpose

        with (
            tc.tile_pool(name="xs2p", bufs=1) as xs2p,
            tc.tile_pool(name="xtp", bufs=2) as xtp,
            tc.tile_pool(name="tfp", bufs=2, space="PSUM") as tfp,
            tc.tile_pool(name="gates", bufs=2, space="PSUM") as gates,
            tc.tile_pool(name="gato", bufs=1, space="PSUM") as gato,
        ):
            # ---- feature transform: xs2[0:64, (t,b,d)] = tanh(Wt@X.T+bt)
            xs2 = xs2p.tile([TS + 1, R], BF16)
            nc.sync.dma_start(xs2[TS : TS + 1, :], ONES)
            xt_tiles = {}

            def feature_round(rr):
                # 512-col round (1 PSUM bank); one big DMA per 4 rounds
                if rr % 4 == 0 and rr // 4 not in xt_tiles:
                    xt = xtp.tile([NF, 2048], BF16, tag="xt", name=f"xt{rr}")
                    nc.sync.dma_start(xt, XT[:, rr * 512 : rr * 512 + 2048])
                    xt_tiles[rr // 4] = xt
                xt = xt_tiles[rr // 4]
                pz = tfp.tile([TS, 512], F32, tag="pz", name=f"pz{rr}")
                nc.tensor.matmul(
                    pz, lhsT=wtt, rhs=xt[:, (rr % 4) * 512 : (rr % 4 + 1) * 512],
                    start=True, stop=True,
                )
                nc.scalar.activation(
                    out=xs2[0:TS, rr * 512 : (rr + 1) * 512],
                    in_=pz, func=AF.Tanh, bias=bt, scale=1.0,
                )


            # rounds 0-5 up front; the rest interleave into early loop steps.
            # round r covers t in [4r, 4r+4) and must be done before the xW
            # prefetch of step 4r-2 touches it.
            feature_round(0)
            nc.sync.dma_start(wih, WIH)
            nc.sync.dma_start(whh, WHH)
            for rr in range(1, 6):
                feature_round(rr)
            feat_sched = {4 * r - 8: r for r in range(6, R // 512)}

            # ---- recurrence ----
            h_prev = [None, None]
            c_prev = [None, None]
            for d in range(2):
                h0 = hpool.tile([H, J], BF16, tag=f"h{d}", name=f"hz{d}")
                nc.vector.memset(h0, 0.0)
                c0 = cpool.tile([H, J], BF16, tag=f"c{d}", name=f"cz{d}")
                nc.vector.memset(c0, 0.0)
                h_prev[d] = h0
                c_prev[d] = c0

            if True:

                def emit_xw(tg):
                    """xW matmuls for step tg.  Per dir: a 3-gate (i,f,g)
                    tile plus a separate o-gate tile, so the big sigmoid's
                    tile-level dependency clears one hW matmul earlier."""
                    tiles = [None, None]
                    for d in range(2):
                        pg = gates.tile([H, 3, J], F32, tag=f"g{d}",
                                        name=f"pg{d}_{tg}")
                        po = gato.tile([H, 1, J], F32, tag=f"o{d}",
                                      name=f"po{d}_{tg}")
                        tiles[d] = (pg, po)
                        rhs_x = xs2[:, tg * J : (tg + 1) * J]
                        for c in range(4):
                            dst = pg[:, c, :] if c < 3 else po[:, 0, :]
                            nc.tensor.matmul(
                                dst,
                                lhsT=wih[:, d, c * H : (c + 1) * H],
                                rhs=rhs_x, start=(c == 0 or c == 3),
                                stop=False,
                            )
                    return tiles

                nxt = emit_xw(0)
                hst = None
                for t in range(T):
                    if t % SB == 0:
                        # [h, s, dir, j] staging slab for this 8-step group
                        hst = hstp.tile([H, SB, 2, J], BF16, tag="hst",
                                        name=f"hst{t // SB}")
                    psg = nxt
                    # hW matmuls for this step (both dirs); the (i,f,g) tile
                    # closes at its own third matmul, the o-gate trails.
                    for d in range(2):
                        pg, po = psg[d]
                        for c in range(4):
                            dst = pg[:, c, :] if c < 3 else po[:, 0, :]
                            nc.tensor.matmul(
                                dst,
                                lhsT=whh[:, d, c * H : (c + 1) * H],
                                rhs=h_prev[d], start=False,
                                stop=(c == 2 or c == 3),
                            )
                    # prefetch next step's xW right behind this step's hW;
                    # remaining feature rounds ride the loop's idle slack
                    if t + 1 < T:
                        nxt = emit_xw(t + 1)
                    if t in feat_sched:
                        feature_round(feat_sched[t])
                    # activations first (Act queue order: sig f, sig b)
                    sg = [None, None]
                    for d in range(2):
                        sg[d] = sgpool.tile([H, 4, J], BF16, tag=f"sg{d}",
                                            name=f"sg{d}")
                        nc.scalar.activation(
                            out=sg[d][:, 0:3, :], in_=psg[d][0][:, 0:3, :],
                            func=AF.Sigmoid,
                        )
                    for d in range(2):
                        nc.scalar.activation(
                            out=sg[d][:, 3, :], in_=psg[d][1][:, 0, :],
                            func=AF.Sigmoid,
                        )
                    # c-chains on DVE
                    # cell state tracked as S = c/2, so the update is a plain
                    # 2x-mode tensor add and tanh(c) = tanh(2S) rides the
                    # activation's free input scale.
                    cn_ = [None, None]
                    for d in range(2):
                        c2 = small.tile([H, J], BF16, tag=f"c2{d}", name=f"c2{d}")
                        nc.vector.tensor_mul(c2, sg[d][:, 1, :], c_prev[d])
                        u2 = small.tile([H, J], BF16, tag=f"u2{d}", name=f"u2{d}")
                        nc.vector.scalar_tensor_tensor(
                            u2, sg[d][:, 2, :], -0.5, sg[d][:, 0, :],
                            op0=ALU.add, op1=ALU.mult,
                        )
                        cn = cpool.tile([H, J], BF16, tag=f"c{d}", name=f"cn{d}")
                        nc.vector.tensor_add(cn, u2, c2)
                        cn_[d] = cn
                    # tanh(2S) (Act queue: th f, th b), then hn on DVE
                    th_ = [None, None]
                    for d in range(2):
                        th_[d] = small.tile([H, J], BF16, tag=f"th{d}",
                                            name=f"th{d}")
                        nc.scalar.activation(out=th_[d], in_=cn_[d],
                                             func=AF.Tanh, scale=2.0)
                    for d in range(2):
                        hn = hst[:, t % SB, d, :]
                        nc.vector.tensor_mul(hn, sg[d][:, 3, :], th_[d])
                        h_prev[d] = hn
                        c_prev[d] = cn_[d]
                    # one batched xbar transpose per SB steps: slab
                    # [H, SB*2*J] -> ht[:, t0:t0+SB, :] as [J, SB*2, H]
                    if t % SB == SB - 1:
                        t0 = t - (SB - 1)
                        dst = ht[:, t0 : t0 + SB, :].rearrange(
                            "j s (d h) -> j (s d) h", d=2
                        )
                        nc.sync.dma_start_transpose(dst, hst)

        # ---- tail: attention pooling + context norm ----
        with (
            tc.tile_pool(name="tailp", bufs=1) as tailp,
            tc.tile_pool(name="tailps", bufs=1, space="PSUM") as tailps,
        ):
            htj = ht[:, T - 1, :]  # [J, 2H] last hidden state
            htj_b = bass.AP(
                tensor=htj.tensor, offset=htj.offset,
                ap=[list(htj.ap[0]), [0, T], list(htj.ap[-1])],
            )
            prod = tailp.tile([J, T, 2 * H], BF16)
            nc.vector.tensor_mul(prod, ht, htj_b)
            # pairwise-tree sum over h: bf16 levels ping-pong {pp0, prod}
            pp0 = tailp.tile([J, T, 128], BF16)
            nc.vector.tensor_add(pp0, prod[:, :, 0:128], prod[:, :, 128:256])
            nc.vector.tensor_add(prod[:, :, 0:64], pp0[:, :, 0:64], pp0[:, :, 64:128])
            nc.vector.tensor_add(pp0[:, :, 0:32], prod[:, :, 0:32], prod[:, :, 32:64])
            nc.vector.tensor_add(prod[:, :, 0:16], pp0[:, :, 0:16], pp0[:, :, 16:32])
            nc.vector.tensor_add(pp0[:, :, 0:8], prod[:, :, 0:8], prod[:, :, 8:16])
            ltrf = tailp.tile([J, T, 4], F32)
            nc.vector.tensor_add(ltrf, pp0[:, :, 0:4], pp0[:, :, 4:8])
            w = 4
            while w > 1:
                w //= 2
                nc.vector.tensor_add(ltrf[:, :, 0:w], ltrf[:, :, 0:w],
                                     ltrf[:, :, w : 2 * w])
            logits = ltrf[:, :, 0:1].rearrange("j t one -> j (t one)")
            mx = tailp.tile([J, 1], F32)
            nc.vector.tensor_reduce(mx, logits, axis=mybir.AxisListType.X, op=ALU.max)
            mxn = tailp.tile([J, 1], F32)
            nc.vector.tensor_scalar_mul(mxn, mx, -1.0)
            ew = tailp.tile([J, T], F32)
            dsum = tailp.tile([J, 1], F32)
            nc.scalar.activation(out=ew, in_=logits, func=AF.Exp, bias=mxn,
                                 scale=1.0, accum_out=dsum)
            rd = tailp.tile([J, 1], F32)
            nc.vector.reciprocal(rd, dsum)
            nc.vector.tensor_scalar_mul(ew, ew, rd)  # softmax weights in place
            prod2 = tailp.tile([J, T, 2 * H], BF16, tag="prod")  # reuse slab
            # ew[j,t] is a per-partition scalar for fixed t.  First 64 t's:
            # plain weighted copies (split DVE/Scalar); second 64 t's fold
            # the tree's first level in: prod2[p] += ew[p+64]*ht[p+64].
            for tt in range(64):
                if tt % 5 == 4:
                    nc.vector.tensor_scalar_mul(prod2[:, tt, :], ht[:, tt, :],
                                                ew[:, tt : tt + 1])
                else:
                    nc.scalar.activation(out=prod2[:, tt, :], in_=ht[:, tt, :],
                                         func=AF.Copy, scale=ew[:, tt : tt + 1])
            for tt in range(64, T):
                nc.vector.scalar_tensor_tensor(
                    prod2[:, tt - 64, :], ht[:, tt, :], ew[:, tt : tt + 1],
                    prod2[:, tt - 64, :], op0=ALU.mult, op1=ALU.add,
                )
            # preload the sqrt table set while DVE runs the tree below; the
            # Copy ops above ran from the exp set, and everything after this
            # point (Square, Sqrt) lives in the sqrt set too.
            sqrt_warm = tailp.tile([1, 1], F32)
            nc.scalar.activation(out=sqrt_warm, in_=mx[0:1, :], func=AF.Sqrt)
            # pairwise-tree sum over t (prod2[0:64] holds pair sums already)
            qq = pp0.rearrange("j a b -> j (a b)").rearrange(
                "j (a b) -> j a b", a=64)
            nc.vector.tensor_add(qq[:, 0:32, :], prod2[:, 0:32, :],
                                 prod2[:, 32:64, :])
            nc.vector.tensor_add(prod2[:, 0:16, :], qq[:, 0:16, :],
                                 qq[:, 16:32, :])
            nc.vector.tensor_add(qq[:, 0:8, :], prod2[:, 0:8, :],
                                 prod2[:, 8:16, :])
            nc.vector.tensor_add(prod2[:, 0:4, :], qq[:, 0:4, :], qq[:, 4:8, :])
            ptrf = tailp.tile([J, 2, 2 * H], F32)
            nc.vector.tensor_add(ptrf, prod2[:, 0:2, :], prod2[:, 2:4, :])
            nc.vector.tensor_add(ptrf[:, 0:1, :], ptrf[:, 0:1, :],
                                 ptrf[:, 1:2, :])
            pooled = ptrf[:, 0:1, :].rearrange("j one p -> j (one p)")

            # context norm across each sample's (d, 2h) block
            pooled2 = tailp.tile([J, 2 * H], F32)
            nc.vector.tensor_mul(pooled2, pooled, pooled)
            sel = tailp.tile([J, BLOC], F32)
            nc.sync.dma_start(sel, ins["SEL"])
            pstat = tailps.tile([BLOC, 2 * G4], F32, tag="stats")
            nc.tensor.matmul(pstat[:, 0 : 2 * H], lhsT=sel, rhs=pooled,
                             start=True, stop=False)
            nc.tensor.matmul(pstat[:, 2 * H : 4 * H], lhsT=sel, rhs=pooled2,
                             start=False, stop=True)
            s1 = tailp.tile([BLOC, 1], F32)
            nc.vector.tensor_reduce(s1, pstat[:, 0 : 2 * H],
                                    axis=mybir.AxisListType.X, op=ALU.add)
            s2 = tailp.tile([BLOC, 1], F32)
            nc.vector.tensor_reduce(s2, pstat[:, 2 * H : 4 * H],
                                    axis=mybir.AxisListType.X, op=ALU.add)
            stats2 = tailp.tile([BLOC, 2], F32)
            nc.scalar.mul(stats2[:, 0:1], s1, 1.0 / NORM_N)      # mean
            q = tailp.tile([BLOC, 1], F32)
            nc.vector.tensor_mul(q, s1, stats2[:, 0:1])          # sum*mean
            v = tailp.tile([BLOC, 1], F32)
            nc.vector.tensor_tensor(v, s2, q, op=ALU.subtract)
            sd = tailp.tile([BLOC, 1], F32)
            nc.scalar.activation(out=sd, in_=v, func=AF.Sqrt,
                                 scale=1.0 / (NORM_N - 1))
            nc.vector.reciprocal(stats2[:, 1:2], sd)             # rstd
            selt = tailp.tile([BLOC, J], F32)
            nc.sync.dma_start(selt, ins["SELT"])
            pmb = tailps.tile([J, 2], F32, tag="mb")
            nc.tensor.matmul(pmb, lhsT=selt, rhs=stats2, start=True, stop=True)
            mb = tailp.tile([J, 2], F32)
            nc.vector.tensor_copy(mb, pmb)
            dwt = tailp.tile([J, 2 * H], F32)
            nc.sync.dma_start(dwt[0:D, :], DW)
            nc.sync.dma_start(dwt[D:J, :], DW)
            dbt = tailp.tile([J, 2 * H], F32)
            nc.sync.dma_start(dbt[0:D, :], DB)
            nc.sync.dma_start(dbt[D:J, :], DB)
            t1 = tailp.tile([J, 2 * H], F32)
            nc.vector.tensor_scalar(t1, pooled, mb[:, 0:1], mb[:, 1:2],
                                    op0=ALU.subtract, op1=ALU.mult)
            t2 = tailp.tile([J, 2 * H], F32)
            nc.vector.tensor_mul(t2, t1, dwt)
            t3 = tailp.tile([J, 2 * H], F32)
            nc.vector.tensor_add(t3, t2, dbt)
            nc.sync.dma_start(OUT, t3)


def build_program():
    nc = bacc.Bacc("TRN2", target_bir_lowering=False, debug=False)
    ins = {
        "XT": nc.dram_tensor("XT", [NF, R], BF16, kind="ExternalInput").ap(),
        "WTT": nc.dram_tensor("WTT", [NF, TS], BF16, kind="ExternalInput").ap(),
        "BT": nc.dram_tensor("BT", [TS, 1], F32, kind="ExternalInput").ap(),
        "WIH": nc.dram_tensor("WIH", [TS + 1, 2, G4], BF16, kind="ExternalInput").ap(),
        "WHH": nc.dram_tensor("WHH", [H, 2, G4], BF16, kind="ExternalInput").ap(),
        "ONES": nc.dram_tensor("ONES", [1, R], BF16, kind="ExternalInput").ap(),
        "DW": nc.dram_tensor("DW", [D, 2 * H], F32, kind="ExternalInput").ap(),
        "SEL": nc.dram_tensor("SEL", [J, BLOC], F32, kind="ExternalInput").ap(),
        "SELT": nc.dram_tensor("SELT", [BLOC, J], F32, kind="ExternalInput").ap(),
        "DB": nc.dram_tensor("DB", [D, 2 * H], F32, kind="ExternalInput").ap(),
    }
    outs = {
        "OUT": nc.dram_tensor("OUT", [J, 2 * H], F32, kind="ExternalOutput").ap(),
    }
    with tile.TileContext(nc) as tc:
        emit(tc, ins, outs)
    nc.compile()
    return nc


def _prep_dir(Wih, Whh, bih, bhh):
    # gate order (i,f,o,g); the g block is pre-scaled by 2 so the kernel can
    # evaluate tanh(g) as 2*sigmoid(2g)-1 inside the fused sigmoid op
    wihT = Wih.T.reshape(TS, 4, H)[:, PERM, :].reshape(TS, G4).copy()
    biasr = (bih + bhh).reshape(4, H)[PERM, :].reshape(G4).copy()
    wihT[:, 2 * H : 3 * H] *= 2.0
    biasr[2 * H : 3 * H] *= 2.0
    wih65 = np.concatenate([wihT, biasr[None, :]], axis=0).astype(BF16NP)
    whhT = Whh.T.reshape(H, 4, H)[:, PERM, :].reshape(H, G4).copy()
    whhT[:, 2 * H : 3 * H] *= 2.0
    whhT = whhT.astype(BF16NP)
    return wih65, whhT


def prep_inputs(X, W_t, b_t, Wih_f, Whh_f, bih_f, bhh_f,
                Wih_b, Whh_b, bih_b, bhh_b, diag_w, diag_b):
    wih_f, whh_f = _prep_dir(Wih_f, Whh_f, bih_f, bhh_f)
    wih_b, whh_b = _prep_dir(Wih_b, Whh_b, bih_b, bhh_b)
    shared = {
        "WTT": np.ascontiguousarray(W_t.T, dtype=BF16NP),
        "BT": np.ascontiguousarray(b_t.reshape(TS, 1), dtype=np.float32),
        "WIH": np.ascontiguousarray(np.stack([wih_f, wih_b], axis=1)),
        "WHH": np.ascontiguousarray(np.stack([whh_f, whh_b], axis=1)),
        "ONES": np.ones((1, R), dtype=BF16NP),
        "SEL": np.kron(np.eye(BLOC, dtype=np.float32), np.ones((D, 1), np.float32)),
        "SELT": np.kron(np.eye(BLOC, dtype=np.float32), np.ones((1, D), np.float32)),
        "DW": np.ascontiguousarray(diag_w.reshape(D, 2 * H), dtype=np.float32),
        "DB": np.ascontiguousarray(diag_b.reshape(D, 2 * H), dtype=np.float32),
    }
    in_maps = []
    for i in range(NCORES):
        xt = np.ascontiguousarray(
            X[i * BLOC : (i + 1) * BLOC].transpose(3, 1, 0, 2).reshape(NF, R),
            dtype=BF16NP,
        )
        m = {"XT": xt}
        m.update(shared)
        in_maps.append(m)
    return in_maps


def kernel(**inputs):
    inputs = {k: np.asarray(v, dtype=np.float32) for k, v in inputs.items()}
    in_maps = prep_inputs(**inputs)
    nc = build_program()
    res = run_bass_kernel_spmd(nc, in_maps, list(range(NCORES)))
    out = np.concatenate(
        [res.results[i]["OUT"].reshape(BLOC, D, 2 * H) for i in range(NCORES)],
        axis=0,
    )
    return np.ascontiguousarray(out, dtype=np.float32)


if __name__ == "__main__":
    nc = build_program()
    print("program built ok")

